# revision 6
# baseline (speedup 1.0000x reference)
"""Causal attention (B=4, S=2048, D=1024, single head) on 8 TRN2 NeuronCores.

Sharding: data-parallel over batch (4 pairs of cores); within each pair
the K/V context is split by interleaved 128-row chunks (core parity p
owns global k-chunks {2j+p}).  Each core projects K/V for its own 1024
context rows and Q for its own 1024 rows; the pair exchanges Q halves
with a 2-core AllGather so both cores hold Q for all 2048 rows in a
canonical "gathered" column order ([all even 128-blocks | all odd
128-blocks] — rank-indexed, hence identical on both cores).  Each core
then computes its causal score blocks against its own context and
produces *unnormalized* partial attention output plus the per-row
partial softmax denominator.  The host adds the two partials of each
pair and normalizes.

The SPMD program is identical across cores; all parity-dependent causal
structure lives in input data (per-core column-permuted x, per-core mask
tiles).  All matmuls run in bf16 (fp32 PSUM accumulation); inputs are
pre-cast on the host.
"""

import sys

if "/opt/trn_rl_repo" not in sys.path:
    sys.path.insert(0, "/opt/trn_rl_repo")

import ml_dtypes
import numpy as np

import concourse.bacc as bacc
import concourse.tile as tile
from concourse import mybir
from concourse.bass_utils import run_bass_kernel_spmd

B, S, D = 4, 2048, 1024
NB = S // 128          # 16 q-blocks of 128 per batch
NT = S // 512          # 4 q-tiles of 512
IC = D // 128          # 8 contraction chunks
OC = D // 128          # 8 output-dim chunks
LC = 8                 # local k-chunks per core (S/2/128)
NMSK = 16              # mask tiles: 4 per q-tile
SCALE = 1.0 / np.sqrt(D)  # 0.03125
NJ_TILE = [4, 8, 4, 8]  # local k-chunks needed per gathered q-tile

BF16 = mybir.dt.bfloat16
F32 = mybir.dt.float32

_module_cache = None
last_results = None  # BassKernelResults of the most recent run (for test harness)


def _masked_js(tt):
    """Local chunk indices whose score blocks need a mask for q-tile tt."""
    return range(4) if tt in (0, 2) else range(4, 8)


def _build_module():
    nc = bacc.Bacc("TRN2", target_bir_lowering=False, debug=False, num_devices=8)
    xT = nc.dram_tensor("xT", [D, S], BF16, kind="ExternalInput").ap()
    wqT = nc.dram_tensor("wqT", [D, D], BF16, kind="ExternalInput").ap()
    wkT = nc.dram_tensor("wkT", [D, D], BF16, kind="ExternalInput").ap()
    wvT = nc.dram_tensor("wvT", [D, D], BF16, kind="ExternalInput").ap()
    msk = nc.dram_tensor("msk", [NMSK * 128, 512], BF16, kind="ExternalInput").ap()
    out_p = nc.dram_tensor("out_p", [S, D], F32, kind="ExternalOutput").ap()
    rs_out = nc.dram_tensor("rs_out", [1, S], F32, kind="ExternalOutput").ap()

    with tile.TileContext(nc) as tc:
        with (
            tc.tile_pool(name="wp", bufs=1) as wp,
            tc.tile_pool(name="xp", bufs=1) as xp,
            tc.tile_pool(name="kqv", bufs=1) as kqv,
            tc.tile_pool(name="mp", bufs=1) as mp,
            tc.tile_pool(name="ptp", bufs=2) as ptp,
            tc.tile_pool(name="stg", bufs=4) as stg,
            tc.tile_pool(name="qsg", bufs=4) as qsg,
            tc.tile_pool(name="dr", bufs=1, space="DRAM") as dr,
        ):
            # ---- input DMA (ordered so the K-projection can start ASAP) ----
            wk_sb, xt_sb = [], []
            for i in range(IC):
                tk = wp.tile([128, D], BF16, tag=f"wk{i}", name=f"wk{i}")
                nc.sync.dma_start(tk, wkT[128 * i : 128 * (i + 1), :])
                wk_sb.append(tk)
                tx = xp.tile([128, S], BF16, tag=f"x{i}", name=f"x{i}")
                nc.sync.dma_start(tx[:, 0 : S // 2], xT[128 * i : 128 * (i + 1), 0 : S // 2])
                xt_sb.append(tx)
            wq_sb = []
            for i in range(IC):
                tq = wp.tile([128, D], BF16, tag=f"wq{i}", name=f"wq{i}")
                nc.sync.dma_start(tq, wqT[128 * i : 128 * (i + 1), :])
                nc.sync.dma_start(
                    xt_sb[i][:, S // 2 : S], xT[128 * i : 128 * (i + 1), S // 2 : S]
                )
                wq_sb.append(tq)
            wv_sb = []
            for i in range(IC):
                tv = wp.tile([128, D], BF16, tag=f"wv{i}", name=f"wv{i}")
                nc.sync.dma_start(tv, wvT[128 * i : 128 * (i + 1), :])
                wv_sb.append(tv)
            mask_sb = []
            for m in range(NMSK):
                tm = mp.tile([128, 512], BF16, tag=f"m{m}", name=f"m{m}")
                nc.sync.dma_start(tm, msk[128 * m : 128 * (m + 1), :])
                mask_sb.append(tm)
            ones_sb = mp.tile([128, 1], BF16, tag="ones", name="ones")
            nc.any.memset(ones_sb, 1.0)

            # even-128-block views of x (this core's own k context rows)
            xe = [
                t.rearrange("p (j two r) -> p j two r", two=2, r=128) for t in xt_sb
            ]

            kt_sb = [kqv.tile([128, S // 2], BF16, tag=f"kt{o}", name=f"kt{o}") for o in range(OC)]
            qt_sb = [kqv.tile([128, S], BF16, tag=f"qt{o}", name=f"qt{o}") for o in range(OC)]
            vn_sb = [kqv.tile([128, D], BF16, tag=f"vn{j}", name=f"vn{j}") for j in range(LC)]

            # DRAM bounce buffers for the pairwise Q-half exchange
            qhalf = dr.tile([D, S // 2], BF16, name="qhalf")
            qfull = dr.tile([2 * D, S // 2], BF16, name="qfull")

            # ---- phase 1 projections: i-outer chains across 8 PSUM banks so
            #      the PE starts as soon as the first (w, x) chunks land ----
            def proj_iouter(ps1, lhs_slices, rhs_slices, dsts, pname):
                pps = [
                    ps1.tile([128, 512], F32, tag=f"proj8_{o}", bufs=1, name=f"{pname}{o}")
                    for o in range(len(dsts))
                ]
                for i in range(IC):
                    for o in range(len(dsts)):
                        nc.tensor.matmul(
                            pps[o],
                            lhsT=lhs_slices(i, o),
                            rhs=rhs_slices(i, o),
                            start=(i == 0),
                            stop=(i == IC - 1),
                        )
                for o, dst in enumerate(dsts):
                    dst(pps[o])

            def copy_to(dst):
                return lambda pp: nc.any.tensor_copy(dst, pp)

            def qhalf_send(o, st):
                def run(pp):
                    qs = qsg.tile([128, 512], BF16, tag="qsg", name="qsg")
                    nc.any.tensor_copy(qs, pp)
                    nc.sync.dma_start(
                        qhalf[128 * o : 128 * (o + 1), 512 * st : 512 * (st + 1)], qs
                    )
                return run

            with tc.tile_pool(name="ps1", bufs=1, space="PSUM") as ps1:
                # K projection st=0 (local k cols 0..512)
                proj_iouter(
                    ps1,
                    lambda i, o: wk_sb[i][:, 128 * o : 128 * (o + 1)],
                    lambda i, o: xe[i][:, 0:4, 0, :],
                    [copy_to(kt_sb[o][:, 0:512]) for o in range(OC)],
                    "pk0",
                )
                # Q projection of this core's own half, st=0
                proj_iouter(
                    ps1,
                    lambda i, o: wq_sb[i][:, 128 * o : 128 * (o + 1)],
                    lambda i, o: xe[i][:, 0:4, 0, :],
                    [qhalf_send(o, 0) for o in range(OC)],
                    "pq0",
                )
                proj_iouter(
                    ps1,
                    lambda i, o: wk_sb[i][:, 128 * o : 128 * (o + 1)],
                    lambda i, o: xe[i][:, 4:8, 0, :],
                    [copy_to(kt_sb[o][:, 512:1024]) for o in range(OC)],
                    "pk1",
                )
                proj_iouter(
                    ps1,
                    lambda i, o: wq_sb[i][:, 128 * o : 128 * (o + 1)],
                    lambda i, o: xe[i][:, 4:8, 0, :],
                    [qhalf_send(o, 1) for o in range(OC)],
                    "pq1",
                )

                # pair exchange of Q halves (runs on the collective engine,
                # overlapped with the V projection below)
                nc.gpsimd.collective_compute(
                    kind="AllGather",
                    op=mybir.AluOpType.bypass,
                    replica_groups=[[0, 1], [2, 3], [4, 5], [6, 7]],
                    ins=[qhalf],
                    outs=[qfull],
                )

                for ot in range(2):
                    proj_iouter(
                        ps1,
                        lambda i, j: xe[i][:, j, 0, :],
                        lambda i, j: wv_sb[i][:, 512 * ot : 512 * (ot + 1)],
                        [copy_to(vn_sb[j][:, 512 * ot : 512 * (ot + 1)]) for j in range(LC)],
                        f"pv{ot}",
                    )

            # gathered Q comes back: rank r half -> qt columns [1024r, 1024r+1024)
            for o in range(OC):
                for r in range(2):
                    nc.sync.dma_start(
                        qt_sb[o][:, 1024 * r : 1024 * (r + 1)],
                        qfull[1024 * r + 128 * o : 1024 * r + 128 * (o + 1), :],
                    )

            # ---- phase 2: attention over gathered q-tiles ----
            rs_sb = mp.tile([1, S], F32, tag="rs", name="rs")
            with tc.tile_pool(name="ps2", bufs=2, space="PSUM") as ps:
                for tt in range(NT):
                    nj = NJ_TILE[tt]
                    masked = set(_masked_js(tt))
                    pt_tiles = []
                    for j in range(nj):
                        sp = ps.tile([128, 512], F32, tag="score", bufs=3, name="score")
                        for o in range(OC):
                            nc.tensor.matmul(
                                sp,
                                lhsT=kt_sb[o][:, 128 * j : 128 * (j + 1)],
                                rhs=qt_sb[o][:, 512 * tt : 512 * (tt + 1)],
                                start=(o == 0),
                                stop=(o == OC - 1),
                            )
                        pt = ptp.tile([128, 512], BF16, tag=f"pt{j}", name=f"pt{j}")
                        nc.scalar.activation(
                            pt, sp, mybir.ActivationFunctionType.Exp, scale=SCALE
                        )
                        if j in masked:
                            m = 4 * tt + (j % 4)
                            nc.vector.tensor_mul(pt, pt, mask_sb[m])
                        pt_tiles.append(pt)

                    for qq in range(4):
                        qbg = 4 * tt + qq        # gathered q-block index
                        njs = (qbg % 8) + 1      # causal chunk count in gathered order
                        for ot in range(2):
                            apsum = ps.tile(
                                [128, 512], F32, tag="attn", bufs=4, name="attn"
                            )
                            for j in range(njs):
                                nc.tensor.matmul(
                                    apsum,
                                    lhsT=pt_tiles[j][:, 128 * qq : 128 * (qq + 1)],
                                    rhs=vn_sb[j][:, 512 * ot : 512 * (ot + 1)],
                                    start=(j == 0),
                                    stop=(j == njs - 1),
                                )
                            ost = stg.tile([128, 512], F32, tag="ost", name="ost")
                            nc.any.tensor_copy(ost, apsum)
                            nc.sync.dma_start(
                                out_p[
                                    128 * qbg : 128 * (qbg + 1),
                                    512 * ot : 512 * (ot + 1),
                                ],
                                ost,
                            )
                    # partial softmax denominators: ones^T @ pt accumulated over j
                    rsp = ps.tile([1, 512], F32, tag="rs", bufs=1, name="rsp")
                    for j in range(nj):
                        nc.tensor.matmul(
                            rsp,
                            lhsT=ones_sb,
                            rhs=pt_tiles[j],
                            start=(j == 0),
                            stop=(j == nj - 1),
                        )
                    nc.any.tensor_copy(rs_sb[:, 512 * tt : 512 * (tt + 1)], rsp)

            nc.sync.dma_start(rs_out, rs_sb)

    nc.compile()
    return nc


def _get_module():
    global _module_cache
    if _module_cache is None:
        _module_cache = _build_module()
    return _module_cache


def _gathered_q(p):
    """Global q index for gathered position p (vectorized)."""
    p = np.asarray(p)
    blk = p // 128
    even = blk < 8
    gb = np.where(even, 2 * blk, 2 * (blk - 8) + 1)
    return 128 * gb + p % 128


def _host_masks(par: int) -> np.ndarray:
    """[NMSK*128, 512] bf16 causal masks in gathered q order."""
    out = np.zeros((NMSK * 128, 512), dtype=np.float32)
    k = np.arange(128)[:, None]
    ql = np.arange(512)[None, :]
    for tt in range(NT):
        for idx, j in enumerate(_masked_js(tt)):
            m = 4 * tt + idx
            g = 2 * j + par  # global k-chunk of local chunk j
            q_global = _gathered_q(512 * tt + ql)
            out[128 * m : 128 * (m + 1), :] = (q_global >= 128 * g + k).astype(
                np.float32
            )
    return out.astype(ml_dtypes.bfloat16)


def kernel(x, Wq, Wk, Wv, _trace=False):
    global last_results
    nc = _get_module()

    bf = ml_dtypes.bfloat16
    wqT = np.ascontiguousarray(Wq.T).astype(bf)
    wkT = np.ascontiguousarray(Wk.T).astype(bf)
    wvT = np.ascontiguousarray(Wv.T).astype(bf)
    masks = [_host_masks(0), _host_masks(1)]

    swap16 = np.arange(NB) ^ 1  # adjacent 128-block swap

    in_maps = []
    for c in range(8):
        b, par = c // 2, c % 2
        xTb = x[b].T  # [D, S] f32
        if par == 1:
            xTb = xTb.reshape(D, NB, 128)[:, swap16].reshape(D, S)
        in_maps.append(
            {
                "xT": np.ascontiguousarray(xTb).astype(bf),
                "wqT": wqT,
                "wkT": wkT,
                "wvT": wvT,
                "msk": masks[par],
            }
        )

    kwargs = {}
    if _trace:
        kwargs["trace"] = True
    res = run_bass_kernel_spmd(nc, in_maps, core_ids=list(range(8)), **kwargs)
    last_results = res

    # rows come back in gathered order; gath_row[q] = gathered position of q
    gath_row = np.empty(S, dtype=np.int64)
    gath_row[_gathered_q(np.arange(S))] = np.arange(S)

    out = np.empty((B, S, D), dtype=np.float32)
    for b in range(B):
        rA = res.results[2 * b]
        rB = res.results[2 * b + 1]
        num = rA["out_p"] + rB["out_p"]
        den = rA["rs_out"][0] + rB["rs_out"][0]
        out[b] = (num / den[:, None])[gath_row]
    return out


# revision 7
# speedup vs baseline: 1.0474x; 1.0474x over previous
"""Causal attention (B=4, S=2048, D=1024, single head) on 8 TRN2 NeuronCores.

Sharding: data-parallel over batch (4 pairs of cores); within each pair
the K/V context is split by interleaved 128-row chunks (core parity p
owns global k-chunks {2j+p}).  Each core projects K/V for its own 1024
context rows and Q for its own 1024 rows; the pair exchanges Q halves
with a 2-core AllGather so both cores hold Q for all 2048 rows in a
canonical "gathered" column order ([all even 128-blocks | all odd
128-blocks] — rank-indexed, hence identical on both cores).  Each core
then computes its causal score blocks against its own context and
produces *unnormalized* partial attention output plus the per-row
partial softmax denominator.  The host adds the two partials of each
pair and normalizes.

The SPMD program is identical across cores; all parity-dependent causal
structure lives in input data (per-core column-permuted x, per-core mask
tiles).  All matmuls run in bf16 (fp32 PSUM accumulation); inputs are
pre-cast on the host.
"""

import sys

if "/opt/trn_rl_repo" not in sys.path:
    sys.path.insert(0, "/opt/trn_rl_repo")

import ml_dtypes
import numpy as np

import concourse.bacc as bacc
import concourse.tile as tile
from concourse import mybir
from concourse.bass_utils import run_bass_kernel_spmd

B, S, D = 4, 2048, 1024
NB = S // 128          # 16 q-blocks of 128 per batch
NT = S // 512          # 4 q-tiles of 512
IC = D // 128          # 8 contraction chunks
OC = D // 128          # 8 output-dim chunks
LC = 8                 # local k-chunks per core (S/2/128)
NMSK = 16              # mask tiles: 4 per q-tile
SCALE = 1.0 / np.sqrt(D)  # 0.03125
NJ_TILE = [4, 8, 4, 8]  # local k-chunks needed per gathered q-tile

BF16 = mybir.dt.bfloat16
F32 = mybir.dt.float32

_module_cache = None
last_results = None  # BassKernelResults of the most recent run (for test harness)


def _masked_js(tt):
    """Local chunk indices whose score blocks need a mask for q-tile tt."""
    return range(4) if tt in (0, 2) else range(4, 8)


def _build_module():
    nc = bacc.Bacc("TRN2", target_bir_lowering=False, debug=False, num_devices=8)
    xT = nc.dram_tensor("xT", [D, S], BF16, kind="ExternalInput").ap()
    wqT = nc.dram_tensor("wqT", [D, D], BF16, kind="ExternalInput").ap()
    wkT = nc.dram_tensor("wkT", [D, D], BF16, kind="ExternalInput").ap()
    wvT = nc.dram_tensor("wvT", [D, D], BF16, kind="ExternalInput").ap()
    msk = nc.dram_tensor("msk", [NMSK * 128, 512], BF16, kind="ExternalInput").ap()
    out_p = nc.dram_tensor("out_p", [S, D], F32, kind="ExternalOutput").ap()
    rs_out = nc.dram_tensor("rs_out", [1, S], F32, kind="ExternalOutput").ap()

    with tile.TileContext(nc) as tc:
        with (
            tc.tile_pool(name="wp", bufs=1) as wp,
            tc.tile_pool(name="xp", bufs=1) as xp,
            tc.tile_pool(name="kqv", bufs=1) as kqv,
            tc.tile_pool(name="mp", bufs=1) as mp,
            tc.tile_pool(name="ptp", bufs=2) as ptp,
            tc.tile_pool(name="stg", bufs=4) as stg,
            tc.tile_pool(name="qsg", bufs=4) as qsg,
            tc.tile_pool(name="dr", bufs=1, space="DRAM") as dr,
        ):
            # ---- input DMA (ordered so the K-projection can start ASAP) ----
            wk_sb, xt_sb = [], []
            for i in range(IC):
                tk = wp.tile([128, D], BF16, tag=f"wk{i}", name=f"wk{i}")
                nc.sync.dma_start(tk, wkT[128 * i : 128 * (i + 1), :])
                wk_sb.append(tk)
                tx = xp.tile([128, S], BF16, tag=f"x{i}", name=f"x{i}")
                nc.sync.dma_start(tx[:, 0 : S // 2], xT[128 * i : 128 * (i + 1), 0 : S // 2])
                xt_sb.append(tx)
            wq_sb = []
            for i in range(IC):
                tq = wp.tile([128, D], BF16, tag=f"wq{i}", name=f"wq{i}")
                nc.sync.dma_start(tq, wqT[128 * i : 128 * (i + 1), :])
                nc.sync.dma_start(
                    xt_sb[i][:, S // 2 : S], xT[128 * i : 128 * (i + 1), S // 2 : S]
                )
                wq_sb.append(tq)
            wv_sb = []
            for i in range(IC):
                tv = wp.tile([128, D], BF16, tag=f"wv{i}", name=f"wv{i}")
                nc.sync.dma_start(tv, wvT[128 * i : 128 * (i + 1), :])
                wv_sb.append(tv)
            mask_sb = []
            for m in range(NMSK):
                tm = mp.tile([128, 512], BF16, tag=f"m{m}", name=f"m{m}")
                nc.sync.dma_start(tm, msk[128 * m : 128 * (m + 1), :])
                mask_sb.append(tm)
            ones_sb = mp.tile([128, 1], BF16, tag="ones", name="ones")
            nc.any.memset(ones_sb, 1.0)

            # even-128-block views of x (this core's own k context rows)
            xe = [
                t.rearrange("p (j two r) -> p j two r", two=2, r=128) for t in xt_sb
            ]

            kt_sb = [kqv.tile([128, S // 2], BF16, tag=f"kt{o}", name=f"kt{o}") for o in range(OC)]
            qt_sb = [kqv.tile([128, S], BF16, tag=f"qt{o}", name=f"qt{o}") for o in range(OC)]
            vn_sb = [kqv.tile([128, D], BF16, tag=f"vn{j}", name=f"vn{j}") for j in range(LC)]

            # DRAM bounce buffers for the pairwise Q-half exchange (split in
            # two st-pieces so the exchange pipelines with the projections)
            qhalf = [dr.tile([D, 512], BF16, name=f"qhalf{st}") for st in range(2)]
            qfull = [dr.tile([2 * D, 512], BF16, name=f"qfull{st}") for st in range(2)]

            # ---- phase 1 projections: i-outer chains across 8 PSUM banks so
            #      the PE starts as soon as the first (w, x) chunks land ----
            def proj_iouter(ps1, lhs_slices, rhs_slices, dsts, pname):
                pps = [
                    ps1.tile([128, 512], F32, tag=f"proj8_{o}", bufs=1, name=f"{pname}{o}")
                    for o in range(len(dsts))
                ]
                for i in range(IC):
                    for o in range(len(dsts)):
                        nc.tensor.matmul(
                            pps[o],
                            lhsT=lhs_slices(i, o),
                            rhs=rhs_slices(i, o),
                            start=(i == 0),
                            stop=(i == IC - 1),
                        )
                for o, dst in enumerate(dsts):
                    dst(pps[o])

            def copy_to(dst):
                return lambda pp: nc.any.tensor_copy(dst, pp)

            def qhalf_send(o, st):
                def run(pp):
                    qs = qsg.tile([128, 512], BF16, tag="qsg", name="qsg")
                    nc.any.tensor_copy(qs, pp)
                    nc.sync.dma_start(qhalf[st][128 * o : 128 * (o + 1), :], qs)
                return run

            def q_exchange(st):
                nc.gpsimd.collective_compute(
                    kind="AllGather",
                    op=mybir.AluOpType.bypass,
                    replica_groups=[[0, 1], [2, 3], [4, 5], [6, 7]],
                    ins=[qhalf[st]],
                    outs=[qfull[st]],
                )
                for o in range(OC):
                    for r in range(2):
                        nc.sync.dma_start(
                            qt_sb[o][:, 1024 * r + 512 * st : 1024 * r + 512 * (st + 1)],
                            qfull[st][1024 * r + 128 * o : 1024 * r + 128 * (o + 1), :],
                        )

            with tc.tile_pool(name="ps1", bufs=1, space="PSUM") as ps1:
                # Q projection of this core's own half, st=0, then exchange it
                # while the K/V projections keep the PE busy
                proj_iouter(
                    ps1,
                    lambda i, o: wq_sb[i][:, 128 * o : 128 * (o + 1)],
                    lambda i, o: xe[i][:, 0:4, 0, :],
                    [qhalf_send(o, 0) for o in range(OC)],
                    "pq0",
                )
                proj_iouter(
                    ps1,
                    lambda i, o: wk_sb[i][:, 128 * o : 128 * (o + 1)],
                    lambda i, o: xe[i][:, 0:4, 0, :],
                    [copy_to(kt_sb[o][:, 0:512]) for o in range(OC)],
                    "pk0",
                )
                q_exchange(0)
                proj_iouter(
                    ps1,
                    lambda i, o: wq_sb[i][:, 128 * o : 128 * (o + 1)],
                    lambda i, o: xe[i][:, 4:8, 0, :],
                    [qhalf_send(o, 1) for o in range(OC)],
                    "pq1",
                )
                proj_iouter(
                    ps1,
                    lambda i, o: wk_sb[i][:, 128 * o : 128 * (o + 1)],
                    lambda i, o: xe[i][:, 4:8, 0, :],
                    [copy_to(kt_sb[o][:, 512:1024]) for o in range(OC)],
                    "pk1",
                )
                q_exchange(1)
                for ot in range(2):
                    proj_iouter(
                        ps1,
                        lambda i, j: xe[i][:, j, 0, :],
                        lambda i, j: wv_sb[i][:, 512 * ot : 512 * (ot + 1)],
                        [copy_to(vn_sb[j][:, 512 * ot : 512 * (ot + 1)]) for j in range(LC)],
                        f"pv{ot}",
                    )

            # ---- phase 2: attention over gathered q-tiles ----
            rs_sb = mp.tile([1, S], F32, tag="rs", name="rs")
            with tc.tile_pool(name="ps2", bufs=2, space="PSUM") as ps:
                for tt in (0, 2, 1, 3):
                    nj = NJ_TILE[tt]
                    masked = set(_masked_js(tt))
                    pt_tiles = []
                    for j in range(nj):
                        sp = ps.tile([128, 512], F32, tag="score", bufs=3, name="score")
                        for o in range(OC):
                            nc.tensor.matmul(
                                sp,
                                lhsT=kt_sb[o][:, 128 * j : 128 * (j + 1)],
                                rhs=qt_sb[o][:, 512 * tt : 512 * (tt + 1)],
                                start=(o == 0),
                                stop=(o == OC - 1),
                            )
                        pt = ptp.tile([128, 512], BF16, tag=f"pt{j}", name=f"pt{j}")
                        nc.scalar.activation(
                            pt, sp, mybir.ActivationFunctionType.Exp, scale=SCALE
                        )
                        if j in masked:
                            m = 4 * tt + (j % 4)
                            nc.vector.tensor_mul(pt, pt, mask_sb[m])
                        pt_tiles.append(pt)

                    for qq in range(4):
                        qbg = 4 * tt + qq        # gathered q-block index
                        njs = (qbg % 8) + 1      # causal chunk count in gathered order
                        for ot in range(2):
                            apsum = ps.tile(
                                [128, 512], F32, tag="attn", bufs=4, name="attn"
                            )
                            for j in range(njs):
                                nc.tensor.matmul(
                                    apsum,
                                    lhsT=pt_tiles[j][:, 128 * qq : 128 * (qq + 1)],
                                    rhs=vn_sb[j][:, 512 * ot : 512 * (ot + 1)],
                                    start=(j == 0),
                                    stop=(j == njs - 1),
                                )
                            ost = stg.tile([128, 512], F32, tag="ost", name="ost")
                            nc.any.tensor_copy(ost, apsum)
                            nc.sync.dma_start(
                                out_p[
                                    128 * qbg : 128 * (qbg + 1),
                                    512 * ot : 512 * (ot + 1),
                                ],
                                ost,
                            )
                    # partial softmax denominators: ones^T @ pt accumulated over j
                    rsp = ps.tile([1, 512], F32, tag="rs", bufs=1, name="rsp")
                    for j in range(nj):
                        nc.tensor.matmul(
                            rsp,
                            lhsT=ones_sb,
                            rhs=pt_tiles[j],
                            start=(j == 0),
                            stop=(j == nj - 1),
                        )
                    nc.any.tensor_copy(rs_sb[:, 512 * tt : 512 * (tt + 1)], rsp)

            nc.sync.dma_start(rs_out, rs_sb)

    nc.compile()
    return nc


def _get_module():
    global _module_cache
    if _module_cache is None:
        _module_cache = _build_module()
    return _module_cache


def _gathered_q(p):
    """Global q index for gathered position p (vectorized)."""
    p = np.asarray(p)
    blk = p // 128
    even = blk < 8
    gb = np.where(even, 2 * blk, 2 * (blk - 8) + 1)
    return 128 * gb + p % 128


def _host_masks(par: int) -> np.ndarray:
    """[NMSK*128, 512] bf16 causal masks in gathered q order."""
    out = np.zeros((NMSK * 128, 512), dtype=np.float32)
    k = np.arange(128)[:, None]
    ql = np.arange(512)[None, :]
    for tt in range(NT):
        for idx, j in enumerate(_masked_js(tt)):
            m = 4 * tt + idx
            g = 2 * j + par  # global k-chunk of local chunk j
            q_global = _gathered_q(512 * tt + ql)
            out[128 * m : 128 * (m + 1), :] = (q_global >= 128 * g + k).astype(
                np.float32
            )
    return out.astype(ml_dtypes.bfloat16)


def kernel(x, Wq, Wk, Wv, _trace=False):
    global last_results
    nc = _get_module()

    bf = ml_dtypes.bfloat16
    wqT = np.ascontiguousarray(Wq.T).astype(bf)
    wkT = np.ascontiguousarray(Wk.T).astype(bf)
    wvT = np.ascontiguousarray(Wv.T).astype(bf)
    masks = [_host_masks(0), _host_masks(1)]

    swap16 = np.arange(NB) ^ 1  # adjacent 128-block swap

    in_maps = []
    for c in range(8):
        b, par = c // 2, c % 2
        xTb = x[b].T  # [D, S] f32
        if par == 1:
            xTb = xTb.reshape(D, NB, 128)[:, swap16].reshape(D, S)
        in_maps.append(
            {
                "xT": np.ascontiguousarray(xTb).astype(bf),
                "wqT": wqT,
                "wkT": wkT,
                "wvT": wvT,
                "msk": masks[par],
            }
        )

    kwargs = {}
    if _trace:
        kwargs["trace"] = True
    res = run_bass_kernel_spmd(nc, in_maps, core_ids=list(range(8)), **kwargs)
    last_results = res

    # rows come back in gathered order; gath_row[q] = gathered position of q
    gath_row = np.empty(S, dtype=np.int64)
    gath_row[_gathered_q(np.arange(S))] = np.arange(S)

    out = np.empty((B, S, D), dtype=np.float32)
    for b in range(B):
        rA = res.results[2 * b]
        rB = res.results[2 * b + 1]
        num = rA["out_p"] + rB["out_p"]
        den = rA["rs_out"][0] + rB["rs_out"][0]
        out[b] = (num / den[:, None])[gath_row]
    return out
